# revision 73
# baseline (speedup 1.0000x reference)
"""Phi4 differential flash-attention block on 8 trn2 NeuronCores.

Sharding: 2-way sequence (stride-2 interleave) x 4-way head-pair tensor
parallel. Core c handles seq group g = c % 2 (query rows g::2) and head
group hg = c // 2 (5 differential head pairs, one KV pair). Each core
computes K/V for the full sequence (its KV pair only), Q for its own
rows, flash attention in transposed-score layout (scoresT = [keys, q]),
the differential combine + rmsnorm, and a partial output projection.
The host sums the 4 head-group partials per seq group and adds out_b.

Implementation notes (v3):
  - fp16 matmul path, fp32 PSUM accumulation.
  - K/V projection is pipelined into attention: only seq chunk 0 is
    projected up front; chunks 1-3 are emitted inside attention
    supertiles 0-2 (the chunk DMAs stream while attention computes).
  - score PSUM: [P, s, kb, QT] s-major groups of 2 key blocks, bufs=2.
    The two s halves run as concurrent row-tiled matmuls and so must
    drain into different PSUM banks; double-buffering keeps score
    matmuls from waiting on the exp ACTIVATE.
  - one exp ACTIVATE per group covers both score sets (1024 cols).
  - AV is deferred two groups (pend-2) so it never waits on exp.
  - out-proj fills and rmsnorm transposes are queued as single-matmul
    closures and serviced inside the attention loop to fill PE idle.
  - projection / out-proj / transpose PSUM shares one double-buffered
    pool; out-proj drains on DVE to f16 (host sums partials in f32).
  - softmax denominator via an appended ones-column in V.
  - subln weight and (1 - lambda_init) folded into out_w on the host.
  - rmsnorm rsqrt via DVE-only Newton iteration.
"""
import math
import os

import numpy as np

import concourse.bacc as bacc
import concourse.tile as tile
import concourse.mybir as mybir
from concourse.bass import ds, ts
from concourse.masks import make_identity
from concourse.bass_utils import run_bass_kernel_spmd

f32 = mybir.dt.float32
f16 = mybir.dt.float16
AF = mybir.ActivationFunctionType
OP = mybir.AluOpType

# Problem constants (hardcoded per harness contract)
S, H, NH, NKV, D = 2048, 2560, 40, 4, 64
LAYER_IDX = 17
LAMBDA_INIT = 0.8 - 0.6 * math.exp(-0.3 * LAYER_IDX)
SCALE = 1.0 / math.sqrt(D)
P = 128
HT = H // P            # 20 h-tiles
N_CORES = 8
N_SEQ = 2              # sequence groups (stride-2)
N_HG = 4               # head groups
PAIRS = 5              # head pairs per core
S_LOC = S // N_SEQ     # 1024 own queries per core
QT = 256               # queries per attention supertile
N_JT = S_LOC // QT     # 4
QB = S_LOC // P        # 8 own query blocks
EPS = 1e-5

_PROGRAM = None


def _build_program(sim_compat=False):
    nc = bacc.Bacc()

    # DRAM layouts are partition-major (host pre-permutes) so every DMA
    # needs only one descriptor per partition (descriptor generation on
    # the in-order sync engine costs ~3ns each; [H,S]-style layouts made
    # dma_start issue take multiple microseconds and serialized the
    # stream). Inputs are also SPLIT per seq chunk and DECLARED in the
    # order compute needs them: the runtime stages host inputs into
    # device DRAM at ~400GB/s in declaration order, and each on-chip DMA
    # blocks until its whole source tensor is staged.
    wkvT = nc.dram_tensor("wkvT", [P, HT, 2 * P], f16, kind="ExternalInput")
    bkv = nc.dram_tensor("bkv", [P, 2], f32, kind="ExternalInput")
    bq = nc.dram_tensor("bq", [P, PAIRS], f32, kind="ExternalInput")
    lam = nc.dram_tensor("lam", [1, 1], f32, kind="ExternalInput")
    maskd = nc.dram_tensor("maskd", [P, 2, 4, QT], f16, kind="ExternalInput")
    hkv = [nc.dram_tensor("hkv0", [P, HT, 512], f16, kind="ExternalInput")]
    hqd = [nc.dram_tensor("hqd0", [P, HT, 512], f16, kind="ExternalInput")]
    wqT = nc.dram_tensor("wqT", [P, PAIRS, HT, P], f16, kind="ExternalInput")
    hkv.append(nc.dram_tensor("hkv1", [P, HT, 512], f16,
                              kind="ExternalInput"))
    hqd.append(nc.dram_tensor("hqd1", [P, HT, 512], f16,
                              kind="ExternalInput"))
    hkv.append(nc.dram_tensor("hkv2", [P, HT, 512], f16,
                              kind="ExternalInput"))
    hkv.append(nc.dram_tensor("hkv3", [P, HT, 512], f16,
                              kind="ExternalInput"))
    owT = nc.dram_tensor("owT", [PAIRS, P, H], f16, kind="ExternalInput")
    out = nc.dram_tensor("out", [S_LOC, H], f16, kind="ExternalOutput")

    wkv_v = wkvT[:]                       # [128, ho, 256]
    wq_v = wqT[:]                         # [128, pair, ho, 128]
    ow_v = owT[:]                         # [5, 128, 2560]
    out_v = out[:].rearrange("(qb p) h -> qb p h", p=P)           # [8,128,2560]

    with tile.TileContext(nc) as tc:
        with (
            tc.tile_pool(name="singles", bufs=1) as singles,
            tc.tile_pool(name="hres", bufs=3) as hres,
            tc.tile_pool(name="etile", bufs=3) as etile,
            tc.tile_pool(name="tpool", bufs=2) as tpool,
            tc.tile_pool(name="opool", bufs=4) as opool,
            tc.tile_pool(name="pj", bufs=2, space="PSUM") as pj,
        ):
            # ---------- DMAs: critical first, bulk gated ----------
            # DMA bandwidth is fair-shared across all outstanding
            # transfers, so the phase-1/2 critical set (wkv, hch0, hq0,
            # wq) is issued alone. Bulk prefetches are emitted after a
            # buffer-slot-reuse (WAR) dependency on early compute: the
            # sync engine's in-order wait holds their issue (and every
            # later dma_start) until the critical phase has its data.
            # tiny constants first: bkv gates the first K/V drain
            lam_sb = singles.tile([P, 1], f32)
            nc.sync.dma_start(lam_sb[:], lam[:].partition_broadcast(P))
            bkv_sb = singles.tile([P, 2], f32)
            nc.sync.dma_start(bkv_sb[:], bkv[:])
            bq_sb = singles.tile([P, PAIRS], f32)
            nc.sync.dma_start(bq_sb[:], bq[:])
            wkv_sb = singles.tile([P, HT, 2 * P], f16)
            hch0 = hres.tile([P, HT, 512], f16, name="hch")
            hq0 = hres.tile([P, HT, 512], f16, name="hch")
            wq_sb = singles.tile([P, PAIRS, HT, P], f16)
            hsplit = [(0, 2), (2, 3), (5, 5), (10, 5), (15, 5)]
            for lo, n in hsplit:           # first K/V chunk: tiny subs first
                nc.sync.dma_start(wkv_sb[:, ds(lo, n), :],
                                  wkv_v[:, ds(lo, n), :])
                nc.sync.dma_start(hch0[:, ds(lo, n), :],
                                  hkv[0][:, ds(lo, n), :])
            for lo, n in [(0, 10), (10, 10)]:
                nc.sync.dma_start(hq0[:, ds(lo, n), :],
                                  hqd[0][:, ds(lo, n), :])
            mask_sb = singles.tile([P, 2, 4, QT], f16)
            nc.sync.dma_start(mask_sb[:], maskd[:])
            for p in range(PAIRS):         # Q weights stream per pair
                nc.sync.dma_start(wq_sb[:, p, :, :], wq_v[:, p, :, :])
            hch_r = [hch0, None, None, None]
            hch_r[1] = hres.tile([P, HT, 512], f16, name="hch")
            nc.sync.dma_start(hch_r[1][:], hkv[1][:])
            hq1 = hres.tile([P, HT, 512], f16, name="hch")
            nc.sync.dma_start(hq1[:], hqd[1][:])
            hch_r[2] = hres.tile([P, HT, 512], f16, name="hch")
            nc.sync.dma_start(hch_r[2][:], hkv[2][:])
            hch_r[3] = hres.tile([P, HT, 512], f16, name="hch")
            nc.sync.dma_start(hch_r[3][:], hkv[3][:])
            ow_sb = [singles.tile([P, H], f16, name=f"ow{pt_}")
                     for pt_ in range(PAIRS)]
            for pt_ in range(PAIRS):
                nc.sync.dma_start(ow_sb[pt_][:], ow_v[pt_])

            # ---------- resident constants ----------
            ident = singles.tile([P, P], f16)
            make_identity(nc, ident)

            # ---------- resident activations ----------
            kT = singles.tile([P, S], f16)               # [k1|k2, seq]
            vT = singles.tile([P, S], f16)               # [v1|v2, seq]
            v_sb = singles.tile([P, S // P, 132], f16)   # [keys, kb, v1|v2|1s]
            nc.vector.memset(v_sb[:], 0.0)
            nc.vector.memset(v_sb[:, :, 128:129], 1.0)
            qTall = singles.tile([P, PAIRS, S_LOC], f16)  # [q1|q2, pair, seq]
            xat = [singles.tile([P, QB, P], f32, name=f"xat{p}")
                   for p in range(PAIRS)]                # unnormalized attn rows
            ms_all = singles.tile([P, PAIRS * QB], f32)  # row sum-of-squares
            xhatT = [singles.tile([P, QB, P], f16, name=f"xhatT{p}")
                     for p in range(PAIRS)]

            def kv_chain(sc, f):
                # one K-or-V projection chain for seq chunk sc. The
                # contraction is split into two half-row chains that run as
                # concurrent row-tiled matmuls (like the score pairs) into
                # separate PSUM banks; DVE sums them at drain. The K bias
                # is dropped: it adds a per-query constant to scores, which
                # softmax cancels.
                hch = hch_r[sc]
                pa = pj.tile([P, 512], f32, tag="pj", name="pa")
                pb = pj.tile([P, 512], f32, tag="pj", name="pb")
                for h in range(HT):
                    nc.tensor.matmul(pa[:], wkv_sb[0:64, h, ds(f * P, P)],
                                     hch[0:64, h, :],
                                     start=(h == 0), stop=(h == HT - 1))
                    nc.tensor.matmul(pb[:], wkv_sb[64:128, h, ds(f * P, P)],
                                     hch[64:128, h, :],
                                     start=(h == 0), stop=(h == HT - 1))
                # DVE can read only one PSUM operand: ACT drains half a
                # (folding the V bias), DVE adds half b on top.
                dest = kT if f == 0 else vT
                htmp = tpool.tile([P, 512], f32, name="htmp")
                if f == 0:
                    nc.scalar.copy(htmp[:], pa[:])
                else:
                    nc.scalar.activation(htmp[:], pa[:], AF.Identity,
                                         bias=bkv_sb[:, 1:2])
                nc.vector.tensor_tensor(dest[:, ds(sc * 512, 512)],
                                        htmp[:], pb[:], OP.add)

            def v_transpose2(kb):
                # two 128x128 transposes into one PSUM slot, one strided copy
                pvt = pj.tile([P, 2, P], f16, tag="pj", name="pvt")
                nc.tensor.transpose(pvt[:, 0, :], vT[:, ts(kb, P)], ident[:])
                nc.tensor.transpose(pvt[:, 1, :], vT[:, ts(kb + 1, P)],
                                    ident[:])
                nc.vector.tensor_copy(v_sb[:, ds(kb, 2), 0:128], pvt[:])

            def q_fill(hch, sh, p):
                pa = pj.tile([P, 512], f32, tag="pj", name="pa")
                pb = pj.tile([P, 512], f32, tag="pj", name="pb")
                for h in range(HT):
                    nc.tensor.matmul(pa[:], wq_sb[0:64, p, h, :],
                                     hch[0:64, h, :],
                                     start=(h == 0), stop=(h == HT - 1))
                    nc.tensor.matmul(pb[:], wq_sb[64:128, p, h, :],
                                     hch[64:128, h, :],
                                     start=(h == 0), stop=(h == HT - 1))
                qtmp = tpool.tile([P, 512], f32, name="qtmp")
                nc.scalar.activation(qtmp[:], pa[:], AF.Identity,
                                     bias=bq_sb[:, p:p + 1])
                nc.vector.tensor_tensor(qTall[:, p, ds(sh * 512, 512)],
                                        qtmp[:], pb[:], OP.add)

            # ---------- phase 1: K/V chunk 0 (Q fills stream inside jt0)
            kv_chain(0, 0)
            kv_chain(0, 1)
            v_transpose2(0)
            v_transpose2(2)

            def normalize_newton(qb):
                # batched rmsnorm rsqrt for one query block's 5 row-groups.
                # DVE-only inverse sqrt (bit-trick seed + 2 Newton steps) so
                # the scalar engine never leaves the exp table set.
                csl = ds(qb * PAIRS, PAIRS)
                i32 = mybir.dt.int32
                v = tpool.tile([P, PAIRS], f32, name="vms")
                nc.vector.tensor_scalar(v[:], ms_all[:, csl], 1.0 / P, EPS,
                                        OP.mult, OP.add)
                hv = tpool.tile([P, PAIRS], f32, name="hv")
                nc.vector.tensor_scalar_mul(hv[:], v[:], 0.5)
                fb = tpool.tile([P, PAIRS], f32, name="fb")
                nc.vector.tensor_copy(fb[:], v[:].bitcast(i32))  # int bits -> f32
                nc.vector.tensor_scalar(fb[:], fb[:], -0.5, 1597463007.0,
                                        OP.mult, OP.add)
                yi = tpool.tile([P, PAIRS], i32, name="yi")
                nc.vector.tensor_copy(yi[:], fb[:])              # f32 -> int bits
                y = tpool.tile([P, PAIRS], f32, name="yrs")
                nc.vector.tensor_copy(y[:], yi[:].bitcast(f32))
                t = tpool.tile([P, PAIRS], f32, name="trs")
                for _ in range(2):                               # Newton
                    nc.vector.tensor_tensor(t[:], y[:], y[:], OP.mult)
                    nc.vector.tensor_tensor(t[:], t[:], hv[:], OP.mult)
                    nc.vector.tensor_scalar(t[:], t[:], -1.0, 1.5,
                                            OP.mult, OP.add)
                    nc.vector.tensor_tensor(y[:], y[:], t[:], OP.mult)
                nc.vector.tensor_copy(ms_all[:, csl], y[:])

            def normalize_xhat(qb, p):
                col = qb * PAIRS + p
                xh = tpool.tile([P, P], f16, name="xh")
                nc.vector.tensor_scalar_mul(xh[:], xat[p][:, qb, :],
                                            ms_all[:, col:col + 1])
                pt = pj.tile([P, P], f16, tag="pj", name="pt")
                nc.tensor.transpose(pt[:], xh[:], ident[:])
                nc.vector.tensor_copy(xhatT[p][:, qb, :], pt[:])

            def oproj_drain(po, qb, hc):
                ot = opool.tile([P, 512], f16, name="ot")
                nc.vector.tensor_copy(ot[:], po[:])
                nc.sync.dma_start(out_v[qb][:, ds(hc * 512, 512)], ot[:])

            def oproj_fill(qb, hc):
                po = pj.tile([P, 512], f32, tag="pj", name="po")
                for p in range(PAIRS):
                    nc.tensor.matmul(po[:], xhatT[p][:, qb, :],
                                     ow_sb[p][:, ds(hc * 512, 512)],
                                     start=(p == 0), stop=(p == PAIRS - 1))
                oproj_drain(po, qb, hc)

            # ---- work queue: single-matmul closures serviced inside the
            # attention loop so independent PE work fills exp-wait idle.
            # Items that allocate a pj tile keep their closures contiguous
            # in the queue, so two pj accumulations never interleave.
            wq_items = []

            def push_oproj(qb, hc):
                st = {}

                def mk(p):
                    def go():
                        if p == 0:
                            st["po"] = pj.tile([P, 512], f32, tag="pj",
                                               name="po")
                        nc.tensor.matmul(st["po"][:], xhatT[p][:, qb, :],
                                         ow_sb[p][:, ds(hc * 512, 512)],
                                         start=(p == 0),
                                         stop=(p == PAIRS - 1))
                        if p == PAIRS - 1:
                            oproj_drain(st["po"], qb, hc)
                    return go

                for p in range(PAIRS):
                    wq_items.append(mk(p))

            def push_normalize(qb):
                wq_items.append(lambda: normalize_newton(qb))
                for p in range(PAIRS):
                    wq_items.append(lambda p=p: normalize_xhat(qb, p))

            def qstep(n):
                for _ in range(n):
                    if not wq_items:
                        return
                    wq_items.pop(0)()

            def qdrain():
                while wq_items:
                    wq_items.pop(0)()

            # ---------- phase 3: attention ----------
            with (
                tc.tile_pool(name="psc", bufs=(1 if sim_compat else 2),
                             space="PSUM") as psc,
                tc.tile_pool(name="pav", bufs=1, space="PSUM") as pav,
            ):
                for jt in range(N_JT):
                    E = 4 * (jt + 1)          # key blocks for this supertile
                    NG = E // 2               # 2-key-block score groups
                    if jt > 0:
                        for f in range(10):
                            push_oproj(2 * (jt - 1) + f % 2, f // 2)
                    fsteps = 3 if jt in (1, 2) else 2
                    for p in range(PAIRS):
                        # stream next seq chunk's K/V + later Q rows in at
                        # pair boundaries (DMA arrived during prior work)
                        if jt == 0:
                            q_fill(hq0, 0, p)
                        elif jt == 1:
                            q_fill(hq1, 1, p)
                        if jt < 3:
                            if p == 2:
                                kv_chain(jt + 1, 0)
                            elif p == 3:
                                kv_chain(jt + 1, 1)
                            elif p == 4:
                                v_transpose2(4 * jt + 4)
                                v_transpose2(4 * jt + 6)
                        if sim_compat:
                            avt = [[pav.tile([P, 132], f32, tag=f"av{s}{q}",
                                             name=f"av{s}{q}") for q in range(2)]
                                   for s in range(2)]

                            def avap(s, qs, lo, n, avt=avt):
                                return avt[s][qs][:, ds(lo, n)]
                        else:
                            avt = [pav.tile([P, 264], f32, tag=f"av{s}",
                                            name=f"av{s}") for s in range(2)]

                            def avap(s, qs, lo, n, avt=avt):
                                return avt[s][:, ds(132 * qs + lo, n)]

                        def emit_av(ev, kbs):
                            for j, kb in enumerate(kbs):
                                for qs in range(2):
                                    st = kb == 0 and (qs == 0 or sim_compat)
                                    sp = kb == E - 1
                                    for s in range(2):
                                        nc.tensor.matmul(
                                            avap(s, qs, 0, 132),
                                            ev[:, s, j, ds(128 * qs, 128)],
                                            v_sb[:, kb, :], start=st, stop=sp)

                        pend = []
                        for grp in range(NG):     # 2 key blocks per group
                            kbs = (2 * grp, 2 * grp + 1)
                            # s-major layout: the two s halves run as
                            # concurrent row-tiled matmuls, so they must
                            # drain into different PSUM banks.
                            sct = psc.tile([P, 2, 2, QT], f32, tag="sc",
                                           name="sct")
                            rhs1 = qTall[0:64, p, ds(jt * QT, QT)]
                            rhs2 = qTall[64:128, p, ds(jt * QT, QT)]
                            for j, kb in enumerate(kbs):
                                nc.tensor.matmul(sct[:, 0, j, :],
                                                 kT[0:64, ts(kb, P)],
                                                 rhs1, start=True, stop=True)
                                nc.tensor.matmul(sct[:, 1, j, :],
                                                 kT[64:128, ts(kb, P)],
                                                 rhs2, start=True, stop=True)
                            et = etile.tile([P, 2, 2, QT], f16, name="et")
                            nc.scalar.activation(et[:], sct[:], AF.Exp,
                                                 scale=SCALE)
                            if grp >= NG - 2:     # last 2 groups = diag band
                                dl = grp - (NG - 2)
                                nc.vector.tensor_tensor(
                                    et[:], et[:], mask_sb[:, :, ds(2 * dl, 2), :],
                                    OP.mult)
                            qstep(fsteps)
                            if len(pend) == 2:
                                emit_av(*pend.pop(0))
                            pend.append((et, kbs))
                        for item in pend:
                            emit_av(*item)
                        # epilogue: x~ = av1 - (lam*den1/den2)*av2 is a
                        # per-query multiple of the true diff, and rmsnorm
                        # is scale-invariant (eps negligible), so only the
                        # den ratio is needed. Square+rowsum fused.
                        rho = tpool.tile([P, 2], f32, name="rho")
                        for qs in range(2):
                            nc.vector.reciprocal(rho[:, qs:qs + 1],
                                                 avap(1, qs, 128, 1))
                            nc.vector.tensor_tensor(
                                rho[:, qs:qs + 1], rho[:, qs:qs + 1],
                                avap(0, qs, 128, 1), OP.mult)
                        nc.vector.tensor_scalar_mul(rho[:], rho[:], lam_sb[:])
                        for qs in range(2):
                            qb = jt * 2 + qs
                            xs = xat[p][:, qb, :]
                            xb = tpool.tile([P, P], f32, name="xb")
                            nc.vector.tensor_scalar_mul(
                                xb[:], avap(1, qs, 0, P), rho[:, qs:qs + 1])
                            nc.vector.tensor_tensor(xs, avap(0, qs, 0, P),
                                                    xb[:], OP.subtract)
                            sq = tpool.tile([P, P], f32, name="sq")
                            nc.vector.tensor_tensor(sq[:], xs, xs, OP.mult)
                            col = qb * PAIRS + p
                            nc.vector.reduce_sum(ms_all[:, col:col + 1], sq[:],
                                                 axis=mybir.AxisListType.X)
                    qdrain()
                    if jt < N_JT - 1:
                        push_normalize(jt * 2)
                        push_normalize(jt * 2 + 1)
                    else:
                        # tail: qb6 fills interleave with qb7's normalize
                        normalize_newton(6)
                        for p in range(PAIRS):
                            normalize_xhat(6, p)
                        wq_items.append(lambda: normalize_newton(7))
                        for hc in range(H // 512):
                            push_oproj(6, hc)
                            wq_items.append(
                                lambda p=hc: normalize_xhat(7, p))
                        qdrain()
                        for hc in range(H // 512):
                            oproj_fill(7, hc)

    nc.compile()
    return nc


def _prep_inputs(hidden_states, Wqkv_w, Wqkv_b, out_w, out_b,
                 lambda_q1, lambda_k1, lambda_q2, lambda_k2, subln_w):
    hs = np.asarray(hidden_states, np.float32).reshape(S, H)
    Wqkv_w = np.asarray(Wqkv_w, np.float32)
    Wqkv_b = np.asarray(Wqkv_b, np.float32)
    out_w = np.asarray(out_w, np.float32)
    subln_w = np.asarray(subln_w, np.float32)

    lam_full = np.float32(
        np.exp(np.dot(np.asarray(lambda_q1, np.float64),
                      np.asarray(lambda_k1, np.float64)))
        - np.exp(np.dot(np.asarray(lambda_q2, np.float64),
                        np.asarray(lambda_k2, np.float64)))
        + LAMBDA_INIT)

    # partition-major DRAM layouts, one tensor per seq chunk: each DMA is
    # one contiguous descriptor per partition, and chunks stage in order
    hsT = hs.T.astype(np.float16)                                   # [H, S]
    hkv = [np.ascontiguousarray(
        hsT[:, sc * 512:(sc + 1) * 512].reshape(HT, P, 512)
        .transpose(1, 0, 2)) for sc in range(4)]                    # [128,20,512]
    hidT_q = [[np.ascontiguousarray(
        hs[g::2].T.astype(np.float16)[:, sh * 512:(sh + 1) * 512]
        .reshape(HT, P, 512).transpose(1, 0, 2)) for sh in range(2)]
        for g in range(N_SEQ)]

    masks = []
    kk = np.arange(P)[:, None, None]
    bb = np.arange(4)[None, :, None]
    ii = np.arange(QT)[None, None, :]
    for g in range(N_SEQ):
        m = ((2 * ii + g) >= (128 * bb + kk)).astype(np.float16)    # [128,4,256]
        masks.append(np.ascontiguousarray(
            np.repeat(m[:, None, :, :], 2, axis=1)))                # [128,2,4,256]

    in_maps = []
    for c in range(N_CORES):
        g, hg = c % N_SEQ, c // N_SEQ
        kp = hg // 2                                 # kv pair for this head group
        krows = slice(H + P * kp, H + P * (kp + 1))
        vrows = slice(H + NKV * D + P * kp, H + NKV * D + P * (kp + 1))
        qrows = slice(640 * hg, 640 * (hg + 1))
        wq = np.ascontiguousarray(
            Wqkv_w[qrows].T.astype(np.float16)
            .reshape(HT, P, PAIRS, P).transpose(1, 2, 0, 3))        # [128,5,20,128]
        wkv = np.ascontiguousarray(np.concatenate(
            [Wqkv_w[krows].T, Wqkv_w[vrows].T], axis=1).astype(np.float16)
            .reshape(HT, P, 2 * P).transpose(1, 0, 2))              # [128,20,256]
        bkv = np.ascontiguousarray(
            np.stack([Wqkv_b[krows], Wqkv_b[vrows]], axis=1))            # [128,2]
        bq = np.ascontiguousarray(Wqkv_b[qrows].reshape(PAIRS, P).T)     # [128,5]
        sub = np.tile(subln_w, PAIRS) * (1.0 - LAMBDA_INIT)              # [640]
        ow = np.ascontiguousarray(
            (out_w[:, qrows].T * sub[:, None]).astype(np.float16)
            .reshape(PAIRS, P, H))                                  # [5,128,2560]
        in_maps.append({
            "hkv0": hkv[0], "hkv1": hkv[1], "hkv2": hkv[2], "hkv3": hkv[3],
            "hqd0": hidT_q[g][0], "hqd1": hidT_q[g][1],
            "wkvT": wkv,
            "wqT": wq,
            "owT": ow,
            "bkv": bkv,
            "bq": bq,
            "maskd": masks[g],
            "lam": np.array([[lam_full]], np.float32),
        })
    return in_maps


def run(inputs, trace=False):
    global _PROGRAM
    if _PROGRAM is None:
        _PROGRAM = _build_program(
            sim_compat=os.environ.get("KSIMCOMPAT", "0") == "1")
    in_maps = _prep_inputs(**inputs)
    res = run_bass_kernel_spmd(_PROGRAM, in_maps,
                               core_ids=list(range(N_CORES)), trace=trace)
    out_b = np.asarray(inputs["out_b"], np.float32)
    full = np.empty((S, H), np.float32)
    for g in range(N_SEQ):
        acc = np.zeros((S_LOC, H), np.float32)
        for hg in range(N_HG):
            acc += res.results[hg * N_SEQ + g]["out"].astype(np.float32)
        full[g::2] = acc + out_b
    return full.reshape(1, S, H), res


def kernel(**inputs):
    return run(inputs, trace=False)[0]
